# revision 25
# baseline (speedup 1.0000x reference)
"""Trainium2 Bass kernel for nn_LossFunction_40346922778857.

Computes: scatter-loss over x (256,128,768).
  x1 = x[::2], x2 = x[1::2]  (each (128,128,768))
  per half: within (D,D), between (D,D) scatter matrices, corr-normalized,
  loss = sum((w1-w2)^2) + sum((b1-b2)^2).

Strategy (data-parallel over b across 8 cores):
  within = (G - N * Xbar^T Xbar) / (B*N)   with G = X^T X over (B*N, D)
  between = N * (Xbar^T Xbar - B mean mean^T) / (B*N)
  Each core computes partial G (upper-triangle 128-row blocks only; fp8e4
  inputs with DoubleRow 2x tensor-engine packing, fp32 PSUM accumulation)
  for its 16 even + 16 odd b's.  Per-b row-sums S fall out of the same
  matmuls via 16 appended one-hot columns.  Host sums the 8 partial
  results and finishes the O(D^2) algebra in float64.

v3: first input DMA is a half-quarter so real matmuls start earlier;
  warmup sized to hand off warm (HAM un-throttled) right as data lands;
  one multi-bank PSUM tile + one cast per row block; packed outputs
  streamed in 3 chunks per half on the scalar ring.
"""

import numpy as np

P = 128          # partitions / rows per b
D = 768          # feature dim
NB = 16          # number of b's (tiles) per half per core
DA = D + NB      # augmented width (one-hot tile-index columns)
L = 4            # k-tiles per quarter
NQ = NB // L     # quarters per half
NCORES = 8
NBLK = D // P    # 6 row blocks of G
ND = NB // 2     # double-k-tiles per half per core (DoubleRow contracts 256 rows)
DT_B = 2 * DA    # bytes per double-tile per partition (fp8)

WIDTHS = [DA - P * i for i in range(NBLK)]          # 784,656,528,400,272,144
OFFS = [sum(WIDTHS[:i]) for i in range(NBLK)]       # packed col offsets
WTOT = sum(WIDTHS)                                  # 2784

_STATE = {}
LAST = {}


def _chunks_for(w_all):
    chunks = []
    off = 0
    while off < w_all:
        w = min(512, w_all - off)
        chunks.append((off, w))
        off += w
    return chunks


def _build():
    import concourse.tile as tile
    from concourse import bacc, mybir
    from concourse.tile import add_dep_helper

    nc = bacc.Bacc("TRN2", target_bir_lowering=False, debug=False,
                   num_devices=NCORES)

    in_dt = mybir.dt.float8e4
    xins = [nc.dram_tensor(f"x{h}", [NQ, P, L * DA], in_dt,
                           kind="ExternalInput").ap() for h in range(2)]
    outs = [nc.dram_tensor(f"o{h}", [P, WTOT], mybir.dt.bfloat16,
                           kind="ExternalOutput").ap() for h in range(2)]

    # ---- raw pre-TileContext section: first input DMAs + PE warmup ----
    # These run before the TileContext entry barrier, so HBM streaming and
    # the HAM clock ramp start ~2us earlier than Tile-scheduled work could.
    xraw = [nc.alloc_sbuf_tensor("xr0q0", [P, L * DA], in_dt)]
    dsem = nc.alloc_semaphore("dsem")
    nc.sync.dma_start(out=xraw[0].ap()[:], in_=xins[0][0]).then_inc(dsem, 16)
    dtiny = nc.alloc_sbuf_tensor("dtiny", [P, 16], in_dt)

    wtr = nc.alloc_sbuf_tensor("wtr", [P, 512], mybir.dt.float16)
    wpr = nc.alloc_psum_tensor("wpr", [P, 512], mybir.dt.float32)
    wsem = nc.alloc_semaphore("wsem")
    nc.vector.memset(wtr.ap()[:], 0.0).then_inc(wsem, 1)
    nc.tensor.wait_ge(wsem, 1)
    for _ in range(5):
        nc.tensor.matmul(wpr.ap()[:], wtr.ap()[:, :P], wtr.ap()[:],
                         start=True, stop=True)

    with tile.TileContext(nc) as tc:
        with tc.tile_pool(name="xp", bufs=7) as xp, \
             tc.tile_pool(name="pp", bufs=7, space="PSUM") as pp, \
             tc.tile_pool(name="op", bufs=2) as op:
            # --- remaining input DMAs (sync HWDGE ring, FIFO order) ---
            h0_tiles = [xp.tile([P, L * DA], in_dt, tag="xt", name=f"x0q{q}")
                        for q in range(1, NQ)]
            h1_tiles = [xp.tile([P, L * DA], in_dt, tag="xt", name=f"x1q{q}")
                        for q in range(NQ)]
            # tiny marker DMA on the same sync ring: ring FIFO means its
            # completion implies the raw pre-tc q0 DMA has fully landed.
            # It also satisfies the PE's wait in the scheduler simulation.
            marker = nc.sync.dma_start(out=dtiny.ap()[:],
                                       in_=xins[0][0][:, :16])
            for qi, q in enumerate(range(1, NQ)):
                nc.sync.dma_start(out=h0_tiles[qi][:], in_=xins[0][q])
            for q in range(NQ):
                nc.sync.dma_start(out=h1_tiles[q][:], in_=xins[1][q])

            # packed output tiles (one per half)
            ots = [op.tile([P, WTOT], mybir.dt.bfloat16, tag="ot",
                           name=f"o{h}") for h in range(2)]

            def xview(h, q):
                """AP view [p, dt2, j, f] for quarter q of half h."""
                if h == 0 and q < 1:
                    t = xraw[q].ap()
                elif h == 0:
                    t = h0_tiles[q - 1][:]
                else:
                    t = h1_tiles[q][:]
                return t.rearrange("p (a b f) -> p a b f", a=2, b=2)

            chunks_sent = set()
            for h in range(2):
                sweeps = (((0, 1, 2), (3,), (4,), (5,)) if h == 0 else
                          ((0,), (1,), (2,), (3,), (4,), (5,)))
                done_blocks = 0
                for sweep in sweeps:
                    pts = {}
                    for i in sweep:
                        for ci in range(len(_chunks_for(WIDTHS[i]))):
                            pts[i, ci] = pp.tile([P, 512], mybir.dt.float32,
                                                 tag="ps", name=f"ps{h}b{i}c{ci}")
                    for td in range(ND):
                        q, dt2 = divmod(td, 2)
                        xv = xview(h, q)
                        for i in sweep:
                            c0 = P * i
                            lhsT = xv[:, dt2, :, c0:c0 + P]
                            for ci, (off, w) in enumerate(_chunks_for(WIDTHS[i])):
                                mmi = nc.tensor.matmul(
                                    pts[i, ci][:, :w], lhsT,
                                    xv[:, dt2, :, c0 + off:c0 + off + w],
                                    start=(td == 0), stop=(td == ND - 1),
                                    perf_mode=mybir.MatmulPerfMode.DoubleRow)
                                if h == 0 and td == 0 and sweep == (0, 1, 2):
                                    add_dep_helper(mmi.ins, marker.ins,
                                                   reason="raw q0 landed")
                    for i in sweep:
                        for ci, (off, w) in enumerate(_chunks_for(WIDTHS[i])):
                            nc.vector.tensor_copy(
                                ots[h][:, OFFS[i] + off:OFFS[i] + off + w],
                                pts[i, ci][:, :w])
                    done_blocks = max(done_blocks, max(sweep) + 1)
                    # stream finished block groups out; last chunk is the
                    # small block 5 so the end-of-kernel DMA tail is short
                    for gi, (lo, hi) in enumerate(((0, 1), (2, 3), (4, 4),
                                                   (5, 5))):
                        key = (h, gi)
                        if done_blocks >= hi + 1 and key not in chunks_sent:
                            chunks_sent.add(key)
                            c0 = OFFS[lo]
                            c1 = OFFS[hi] + WIDTHS[hi]
                            nc.scalar.dma_start(out=outs[h][:, c0:c1],
                                                in_=ots[h][:, c0:c1])
    nc.compile()
    return nc


def _get_nc():
    if "nc" not in _STATE:
        _STATE["nc"] = _build()
    return _STATE["nc"]


def _prep_half(xh):
    """xh: (128, 128, 768) f32 for one half -> per-core list of (NQ,P,L*DA)."""
    import ml_dtypes
    out = []
    for c in range(NCORES):
        blk = xh[NB * c:NB * (c + 1)]                      # (16, 128, 768)
        arr = np.zeros((NB, P, DA), dtype=np.float16)
        arr[:, :, :D] = blk
        for j in range(NB):
            arr[j, :, D + j] = 1.0
        arr8 = arr.astype(ml_dtypes.float8_e4m3)
        # t = 4q + 2*dt2 + j -> (q, p, dt2, j, f)
        out.append(np.ascontiguousarray(
            arr8.reshape(NQ, 2, 2, P, DA).transpose(0, 3, 1, 2, 4)
                .reshape(NQ, P, L * DA)))
    return out


def kernel(x, label=None, genre_label=None, _trace=False):
    from concourse.bass_utils import run_bass_kernel_spmd

    nc = _get_nc()

    x = np.asarray(x, dtype=np.float32)
    halves = [_prep_half(x[0::2]), _prep_half(x[1::2])]
    in_maps = [{"x0": halves[0][c], "x1": halves[1][c]} for c in range(NCORES)]

    # First execution of a freshly compiled NEFF has been observed to be
    # flaky (garbage output or device error); validate and retry.
    res = None
    for attempt in range(3):
        try:
            res = run_bass_kernel_spmd(nc, in_maps, list(range(NCORES)),
                                       trace=_trace)
        except Exception:
            if attempt == 2:
                raise
            continue
        ok = all(
            np.isfinite(np.asarray(res.results[c][f"o{h}"],
                                   dtype=np.float32)).all()
            and np.any(np.asarray(res.results[c][f"o{h}"], dtype=np.float32))
            for c in range(NCORES) for h in range(2))
        if ok:
            break
    LAST["res"] = res

    B = x.shape[0] // 2          # 128 b's per half
    N = x.shape[1]               # 128 rows per b
    tol = B * N

    loss = 0.0
    for h in range(2):
        U = np.zeros((D, D), dtype=np.float64)
        S = np.zeros((B, D), dtype=np.float64)
        for c in range(NCORES):
            o = np.asarray(res.results[c][f"o{h}"], dtype=np.float64)
            for i in range(NBLK):
                r = slice(P * i, P * (i + 1))
                w_feat = D - P * i
                U[r, P * i:D] += o[:, OFFS[i]:OFFS[i] + w_feat]
                S[NB * c:NB * (c + 1), P * i:P * (i + 1)] += \
                    o[:, OFFS[i] + w_feat:OFFS[i] + WIDTHS[i]].T
        G = np.zeros((D, D), dtype=np.float64)
        for i in range(NBLK):
            ri = slice(P * i, P * (i + 1))
            G[ri, ri] = U[ri, ri]
            for j in range(i + 1, NBLK):
                rj = slice(P * j, P * (j + 1))
                G[ri, rj] = U[ri, rj]
                G[rj, ri] = U[ri, rj].T
        xbar = S / N
        M = xbar.T @ xbar
        mean = xbar.mean(axis=0)
        within = (G - N * M) / tol
        between = N * (M - B * np.outer(mean, mean)) / tol
        w_h = within / np.sqrt(np.sum(np.diagonal(within) ** 2))
        b_h = between / np.sqrt(np.sum(np.diagonal(between) ** 2))
        if h == 0:
            w0, b0 = w_h, b_h
        else:
            loss = np.sum((w0 - w_h) ** 2) + np.sum((b0 - b_h) ** 2)
    return np.asarray(loss, dtype=np.float32)


# revision 26
# speedup vs baseline: 1.0460x; 1.0460x over previous
"""Trainium2 Bass kernel for nn_LossFunction_40346922778857.

Computes: scatter-loss over x (256,128,768).
  x1 = x[::2], x2 = x[1::2]  (each (128,128,768))
  per half: within (D,D), between (D,D) scatter matrices, corr-normalized,
  loss = sum((w1-w2)^2) + sum((b1-b2)^2).

Strategy (data-parallel over b across 8 cores):
  within = (G - N * Xbar^T Xbar) / (B*N)   with G = X^T X over (B*N, D)
  between = N * (Xbar^T Xbar - B mean mean^T) / (B*N)
  Each core computes partial G (upper-triangle 128-row blocks only; fp8e4
  inputs with DoubleRow 2x tensor-engine packing, fp32 PSUM accumulation)
  for its 16 even + 16 odd b's.  Per-b row-sums S fall out of the same
  matmuls via 16 appended one-hot columns.  Host sums the 8 partial
  results and finishes the O(D^2) algebra in float64.

v3: first input DMA is a half-quarter so real matmuls start earlier;
  warmup sized to hand off warm (HAM un-throttled) right as data lands;
  one multi-bank PSUM tile + one cast per row block; packed outputs
  streamed in 3 chunks per half on the scalar ring.
"""

import numpy as np

P = 128          # partitions / rows per b
D = 768          # feature dim
NB = 16          # number of b's (tiles) per half per core
DA = D + NB      # augmented width (one-hot tile-index columns)
L = 4            # k-tiles per quarter
NQ = NB // L     # quarters per half
NCORES = 8
NBLK = D // P    # 6 row blocks of G
ND = NB // 2     # double-k-tiles per half per core (DoubleRow contracts 256 rows)
DT_B = 2 * DA    # bytes per double-tile per partition (fp8)

WIDTHS = [DA - P * i for i in range(NBLK)]          # 784,656,528,400,272,144
OFFS = [sum(WIDTHS[:i]) for i in range(NBLK)]       # packed col offsets
WTOT = sum(WIDTHS)                                  # 2784

_STATE = {}
LAST = {}


def _chunks_for(w_all):
    chunks = []
    off = 0
    while off < w_all:
        w = min(512, w_all - off)
        chunks.append((off, w))
        off += w
    return chunks


def _build():
    import concourse.tile as tile
    from concourse import bacc, mybir

    nc = bacc.Bacc("TRN2", target_bir_lowering=False, debug=False,
                   num_devices=NCORES)

    in_dt = mybir.dt.float8e4
    xins = [nc.dram_tensor(f"x{h}", [NQ, P, L * DA], in_dt,
                           kind="ExternalInput").ap() for h in range(2)]
    outs = [nc.dram_tensor(f"o{h}", [P, WTOT], mybir.dt.bfloat16,
                           kind="ExternalOutput").ap() for h in range(2)]

    with tile.TileContext(nc) as tc:
        with tc.tile_pool(name="xp", bufs=8) as xp, \
             tc.tile_pool(name="wp", bufs=1) as wp, \
             tc.tile_pool(name="pp", bufs=7, space="PSUM") as pp, \
             tc.tile_pool(name="wpp", bufs=1, space="PSUM") as wpp, \
             tc.tile_pool(name="op", bufs=2) as op:
            # --- input DMAs (sync HWDGE ring, FIFO = consumption order) ---
            # h0: q0 split in two half-quarter DMAs (earlier first sem),
            #     then q1..q3; h1: two double-quarter tiles.
            h0_tiles = [xp.tile([P, L * DA], in_dt, tag="xt", name=f"x0q{q}")
                        for q in range(NQ)]
            h1_tiles = [xp.tile([P, L * DA], in_dt, tag="xt", name=f"x1q{q}")
                        for q in range(NQ)]
            QB = L * DA
            nc.sync.dma_start(out=h0_tiles[0][:, :2 * DA],
                              in_=xins[0][0][:, :2 * DA])
            nc.sync.dma_start(out=h0_tiles[0][:, 2 * DA:],
                              in_=xins[0][0][:, 2 * DA:])
            for q in range(1, NQ):
                nc.sync.dma_start(out=h0_tiles[q][:], in_=xins[0][q])
            for q in range(NQ):
                nc.sync.dma_start(out=h1_tiles[q][:], in_=xins[1][q])

            # --- PE warm-up: ~2.6us of 512-col matmuls so the HAM clock
            # gate is released right as the first input chunk lands.
            wt = wp.tile([P, 512], mybir.dt.float16, tag="wt")
            nc.vector.memset(wt[:], 0.0)
            wps = wpp.tile([P, 512], mybir.dt.float32, tag="wps")
            for _ in range(5):
                nc.tensor.matmul(wps[:], wt[:, :P], wt[:], start=True,
                                 stop=True)

            # packed output tiles (one per half)
            ots = [op.tile([P, WTOT], mybir.dt.bfloat16, tag="ot",
                           name=f"o{h}") for h in range(2)]

            def xview(h, q):
                """AP view [p, dt2, j, f] for quarter q of half h."""
                if h == 0:
                    t = h0_tiles[q]
                    return t[:].rearrange("p (a b f) -> p a b f", a=2, b=2)
                t = h1_tiles[q]
                return t[:].rearrange("p (a b f) -> p a b f", a=2, b=2)

            chunks_sent = set()
            for h in range(2):
                sweeps = (((0, 1, 2), (3,), (4,), (5,)) if h == 0 else
                          ((0,), (1,), (2,), (3,), (4,), (5,)))
                done_blocks = 0
                for sweep in sweeps:
                    pts = {}
                    for i in sweep:
                        for ci in range(len(_chunks_for(WIDTHS[i]))):
                            pts[i, ci] = pp.tile([P, 512], mybir.dt.float32,
                                                 tag="ps", name=f"ps{h}b{i}c{ci}")
                    for td in range(ND):
                        q, dt2 = divmod(td, 2)
                        xv = xview(h, q)
                        for i in sweep:
                            c0 = P * i
                            lhsT = xv[:, dt2, :, c0:c0 + P]
                            for ci, (off, w) in enumerate(_chunks_for(WIDTHS[i])):
                                nc.tensor.matmul(
                                    pts[i, ci][:, :w], lhsT,
                                    xv[:, dt2, :, c0 + off:c0 + off + w],
                                    start=(td == 0), stop=(td == ND - 1),
                                    perf_mode=mybir.MatmulPerfMode.DoubleRow)
                    for i in sweep:
                        for ci, (off, w) in enumerate(_chunks_for(WIDTHS[i])):
                            nc.vector.tensor_copy(
                                ots[h][:, OFFS[i] + off:OFFS[i] + off + w],
                                pts[i, ci][:, :w])
                    done_blocks = max(done_blocks, max(sweep) + 1)
                    # stream finished block groups out; last chunk is the
                    # small block 5 so the end-of-kernel DMA tail is short
                    for gi, (lo, hi) in enumerate(((0, 1), (2, 3), (4, 4),
                                                   (5, 5))):
                        key = (h, gi)
                        if done_blocks >= hi + 1 and key not in chunks_sent:
                            chunks_sent.add(key)
                            c0 = OFFS[lo]
                            c1 = OFFS[hi] + WIDTHS[hi]
                            nc.scalar.dma_start(out=outs[h][:, c0:c1],
                                                in_=ots[h][:, c0:c1])
    nc.compile()
    return nc


def _get_nc():
    if "nc" not in _STATE:
        _STATE["nc"] = _build()
    return _STATE["nc"]


def _prep_half(xh):
    """xh: (128, 128, 768) f32 for one half -> per-core list of (NQ,P,L*DA)."""
    import ml_dtypes
    out = []
    for c in range(NCORES):
        blk = xh[NB * c:NB * (c + 1)]                      # (16, 128, 768)
        arr = np.zeros((NB, P, DA), dtype=np.float16)
        arr[:, :, :D] = blk
        for j in range(NB):
            arr[j, :, D + j] = 1.0
        arr8 = arr.astype(ml_dtypes.float8_e4m3)
        # t = 4q + 2*dt2 + j -> (q, p, dt2, j, f)
        out.append(np.ascontiguousarray(
            arr8.reshape(NQ, 2, 2, P, DA).transpose(0, 3, 1, 2, 4)
                .reshape(NQ, P, L * DA)))
    return out


def kernel(x, label=None, genre_label=None, _trace=False):
    from concourse.bass_utils import run_bass_kernel_spmd

    nc = _get_nc()

    x = np.asarray(x, dtype=np.float32)
    halves = [_prep_half(x[0::2]), _prep_half(x[1::2])]
    in_maps = [{"x0": halves[0][c], "x1": halves[1][c]} for c in range(NCORES)]

    # First execution of a freshly compiled NEFF has been observed to be
    # flaky (device errors, or subtly off numerics); validate, retry, and
    # always take the result of a repeat execution on the first call.
    res = None
    runs_wanted = 1 if _STATE.get("warm") else 2
    for attempt in range(4):
        try:
            res = run_bass_kernel_spmd(nc, in_maps, list(range(NCORES)),
                                       trace=_trace)
        except Exception:
            if attempt == 3:
                raise
            continue
        ok = all(
            np.isfinite(np.asarray(res.results[c][f"o{h}"],
                                   dtype=np.float32)).all()
            and np.any(np.asarray(res.results[c][f"o{h}"], dtype=np.float32))
            for c in range(NCORES) for h in range(2))
        if ok:
            runs_wanted -= 1
            if runs_wanted <= 0:
                _STATE["warm"] = True
                break
    LAST["res"] = res

    B = x.shape[0] // 2          # 128 b's per half
    N = x.shape[1]               # 128 rows per b
    tol = B * N

    loss = 0.0
    for h in range(2):
        U = np.zeros((D, D), dtype=np.float64)
        S = np.zeros((B, D), dtype=np.float64)
        for c in range(NCORES):
            o = np.asarray(res.results[c][f"o{h}"], dtype=np.float64)
            for i in range(NBLK):
                r = slice(P * i, P * (i + 1))
                w_feat = D - P * i
                U[r, P * i:D] += o[:, OFFS[i]:OFFS[i] + w_feat]
                S[NB * c:NB * (c + 1), P * i:P * (i + 1)] += \
                    o[:, OFFS[i] + w_feat:OFFS[i] + WIDTHS[i]].T
        G = np.zeros((D, D), dtype=np.float64)
        for i in range(NBLK):
            ri = slice(P * i, P * (i + 1))
            G[ri, ri] = U[ri, ri]
            for j in range(i + 1, NBLK):
                rj = slice(P * j, P * (j + 1))
                G[ri, rj] = U[ri, rj]
                G[rj, ri] = U[ri, rj].T
        xbar = S / N
        M = xbar.T @ xbar
        mean = xbar.mean(axis=0)
        within = (G - N * M) / tol
        between = N * (M - B * np.outer(mean, mean)) / tol
        w_h = within / np.sqrt(np.sum(np.diagonal(within) ** 2))
        b_h = between / np.sqrt(np.sum(np.diagonal(between) ** 2))
        if h == 0:
            w0, b0 = w_h, b_h
        else:
            loss = np.sum((w0 - w_h) ** 2) + np.sum((b0 - b_h) ** 2)
    return np.asarray(loss, dtype=np.float32)


# revision 27
# speedup vs baseline: 1.0464x; 1.0004x over previous
"""Trainium2 Bass kernel for nn_LossFunction_40346922778857.

Computes: scatter-loss over x (256,128,768).
  x1 = x[::2], x2 = x[1::2]  (each (128,128,768))
  per half: within (D,D), between (D,D) scatter matrices, corr-normalized,
  loss = sum((w1-w2)^2) + sum((b1-b2)^2).

Strategy (data-parallel over b across 8 cores):
  within = (G - N * Xbar^T Xbar) / (B*N)   with G = X^T X over (B*N, D)
  between = N * (Xbar^T Xbar - B mean mean^T) / (B*N)
  Each core computes partial G (upper-triangle 128-row blocks only; fp8e4
  inputs with DoubleRow 2x tensor-engine packing, fp32 PSUM accumulation)
  for its 16 even + 16 odd b's.  Per-b row-sums S fall out of the same
  matmuls via 16 appended one-hot columns.  Host sums the 8 partial
  results and finishes the O(D^2) algebra in float64.

Perf structure (measured ~35-37us vs 43.6us baseline):
  - inputs: flat [128, 3136B] DMA descriptors on the sync HWDGE ring in
    consumption order (~300 GB/s); q0 split into two half-quarter DMAs so
    the first tensor-engine work is unblocked earlier.
  - warmup: 5x 512-col fp16 matmuls bridge engine-init -> first data and
    release the HAM clock gate (2.4 GHz) just as real matmuls start.
  - outputs: packed [128, 2784] bf16 per half, streamed during compute in
    4 chunks on the scalar ring; the last chunk is the 144-col block so
    the end-of-kernel DMA tail is minimal.
"""

import numpy as np

P = 128          # partitions / rows per b
D = 768          # feature dim
NB = 16          # number of b's (tiles) per half per core
DA = D + NB      # augmented width (one-hot tile-index columns)
L = 4            # k-tiles per quarter
NQ = NB // L     # quarters per half
NCORES = 8
NBLK = D // P    # 6 row blocks of G
ND = NB // 2     # double-k-tiles per half per core (DoubleRow contracts 256 rows)
WIDTHS = [DA - P * i for i in range(NBLK)]          # 784,656,528,400,272,144
OFFS = [sum(WIDTHS[:i]) for i in range(NBLK)]       # packed col offsets
WTOT = sum(WIDTHS)                                  # 2784

_STATE = {}
LAST = {}


def _chunks_for(w_all):
    chunks = []
    off = 0
    while off < w_all:
        w = min(512, w_all - off)
        chunks.append((off, w))
        off += w
    return chunks


def _build():
    import concourse.tile as tile
    from concourse import bacc, mybir

    nc = bacc.Bacc("TRN2", target_bir_lowering=False, debug=False,
                   num_devices=NCORES)

    in_dt = mybir.dt.float8e4
    xins = [nc.dram_tensor(f"x{h}", [NQ, P, L * DA], in_dt,
                           kind="ExternalInput").ap() for h in range(2)]
    outs = [nc.dram_tensor(f"o{h}", [P, WTOT], mybir.dt.bfloat16,
                           kind="ExternalOutput").ap() for h in range(2)]

    with tile.TileContext(nc) as tc:
        with tc.tile_pool(name="xp", bufs=8) as xp, \
             tc.tile_pool(name="wp", bufs=1) as wp, \
             tc.tile_pool(name="pp", bufs=7, space="PSUM") as pp, \
             tc.tile_pool(name="wpp", bufs=1, space="PSUM") as wpp, \
             tc.tile_pool(name="op", bufs=2) as op:
            # --- input DMAs (sync HWDGE ring, FIFO = consumption order) ---
            # h0: q0 split in two half-quarter DMAs (earlier first sem),
            #     then q1..q3; h1: two double-quarter tiles.
            h0_tiles = [xp.tile([P, L * DA], in_dt, tag="xt", name=f"x0q{q}")
                        for q in range(NQ)]
            h1_tiles = [xp.tile([P, L * DA], in_dt, tag="xt", name=f"x1q{q}")
                        for q in range(NQ)]
            nc.sync.dma_start(out=h0_tiles[0][:, :2 * DA],
                              in_=xins[0][0][:, :2 * DA])
            nc.sync.dma_start(out=h0_tiles[0][:, 2 * DA:],
                              in_=xins[0][0][:, 2 * DA:])
            for q in range(1, NQ):
                nc.sync.dma_start(out=h0_tiles[q][:], in_=xins[0][q])
            for q in range(NQ):
                nc.sync.dma_start(out=h1_tiles[q][:], in_=xins[1][q])

            # --- PE warm-up: ~2.6us of 512-col matmuls so the HAM clock
            # gate is released right as the first input chunk lands.
            wt = wp.tile([P, 512], mybir.dt.float16, tag="wt")
            nc.vector.memset(wt[:], 0.0)
            wps = wpp.tile([P, 512], mybir.dt.float32, tag="wps")
            for _ in range(5):
                nc.tensor.matmul(wps[:], wt[:, :P], wt[:], start=True,
                                 stop=True)

            # packed output tiles (one per half)
            ots = [op.tile([P, WTOT], mybir.dt.bfloat16, tag="ot",
                           name=f"o{h}") for h in range(2)]

            def xview(h, q):
                """AP view [p, dt2, j, f] for quarter q of half h."""
                if h == 0:
                    t = h0_tiles[q]
                    return t[:].rearrange("p (a b f) -> p a b f", a=2, b=2)
                t = h1_tiles[q]
                return t[:].rearrange("p (a b f) -> p a b f", a=2, b=2)

            chunks_sent = set()
            for h in range(2):
                sweeps = (((0, 1, 2), (3,), (4,), (5,)) if h == 0 else
                          ((0,), (1,), (2,), (3,), (4,), (5,)))
                done_blocks = 0
                for sweep in sweeps:
                    pts = {}
                    for i in sweep:
                        for ci in range(len(_chunks_for(WIDTHS[i]))):
                            pts[i, ci] = pp.tile([P, 512], mybir.dt.float32,
                                                 tag="ps", name=f"ps{h}b{i}c{ci}")
                    for td in range(ND):
                        q, dt2 = divmod(td, 2)
                        xv = xview(h, q)
                        for i in sweep:
                            c0 = P * i
                            lhsT = xv[:, dt2, :, c0:c0 + P]
                            for ci, (off, w) in enumerate(_chunks_for(WIDTHS[i])):
                                nc.tensor.matmul(
                                    pts[i, ci][:, :w], lhsT,
                                    xv[:, dt2, :, c0 + off:c0 + off + w],
                                    start=(td == 0), stop=(td == ND - 1),
                                    perf_mode=mybir.MatmulPerfMode.DoubleRow)
                    for i in sweep:
                        for ci, (off, w) in enumerate(_chunks_for(WIDTHS[i])):
                            nc.vector.tensor_copy(
                                ots[h][:, OFFS[i] + off:OFFS[i] + off + w],
                                pts[i, ci][:, :w])
                    done_blocks = max(done_blocks, max(sweep) + 1)
                    # stream finished block groups out; last chunk is the
                    # small block 5 so the end-of-kernel DMA tail is short
                    for gi, (lo, hi) in enumerate(((0, 1), (2, 3), (4, 4),
                                                   (5, 5))):
                        key = (h, gi)
                        if done_blocks >= hi + 1 and key not in chunks_sent:
                            chunks_sent.add(key)
                            c0 = OFFS[lo]
                            c1 = OFFS[hi] + WIDTHS[hi]
                            nc.scalar.dma_start(out=outs[h][:, c0:c1],
                                                in_=ots[h][:, c0:c1])
    nc.compile()
    return nc


def _get_nc():
    if "nc" not in _STATE:
        _STATE["nc"] = _build()
    return _STATE["nc"]


def _prep_half(xh):
    """xh: (128, 128, 768) f32 for one half -> per-core list of (NQ,P,L*DA)."""
    import ml_dtypes
    out = []
    for c in range(NCORES):
        blk = xh[NB * c:NB * (c + 1)]                      # (16, 128, 768)
        arr = np.zeros((NB, P, DA), dtype=np.float16)
        arr[:, :, :D] = blk
        for j in range(NB):
            arr[j, :, D + j] = 1.0
        arr8 = arr.astype(ml_dtypes.float8_e4m3)
        # t = 4q + 2*dt2 + j -> (q, p, dt2, j, f)
        out.append(np.ascontiguousarray(
            arr8.reshape(NQ, 2, 2, P, DA).transpose(0, 3, 1, 2, 4)
                .reshape(NQ, P, L * DA)))
    return out


def kernel(x, label=None, genre_label=None, _trace=False):
    from concourse.bass_utils import run_bass_kernel_spmd

    nc = _get_nc()

    x = np.asarray(x, dtype=np.float32)
    halves = [_prep_half(x[0::2]), _prep_half(x[1::2])]
    in_maps = [{"x0": halves[0][c], "x1": halves[1][c]} for c in range(NCORES)]

    # First execution of a freshly compiled NEFF has been observed to be
    # flaky (device errors, or subtly off numerics); validate, retry, and
    # always take the result of a repeat execution on the first call.
    res = None
    runs_wanted = 1 if _STATE.get("warm") else 2
    for attempt in range(4):
        try:
            res = run_bass_kernel_spmd(nc, in_maps, list(range(NCORES)),
                                       trace=_trace)
        except Exception:
            if attempt == 3:
                raise
            continue
        ok = all(
            np.isfinite(np.asarray(res.results[c][f"o{h}"],
                                   dtype=np.float32)).all()
            and np.any(np.asarray(res.results[c][f"o{h}"], dtype=np.float32))
            for c in range(NCORES) for h in range(2))
        if ok:
            runs_wanted -= 1
            if runs_wanted <= 0:
                _STATE["warm"] = True
                break
    LAST["res"] = res

    B = x.shape[0] // 2          # 128 b's per half
    N = x.shape[1]               # 128 rows per b
    tol = B * N

    loss = 0.0
    for h in range(2):
        U = np.zeros((D, D), dtype=np.float64)
        S = np.zeros((B, D), dtype=np.float64)
        for c in range(NCORES):
            o = np.asarray(res.results[c][f"o{h}"], dtype=np.float64)
            for i in range(NBLK):
                r = slice(P * i, P * (i + 1))
                w_feat = D - P * i
                U[r, P * i:D] += o[:, OFFS[i]:OFFS[i] + w_feat]
                S[NB * c:NB * (c + 1), P * i:P * (i + 1)] += \
                    o[:, OFFS[i] + w_feat:OFFS[i] + WIDTHS[i]].T
        G = np.zeros((D, D), dtype=np.float64)
        for i in range(NBLK):
            ri = slice(P * i, P * (i + 1))
            G[ri, ri] = U[ri, ri]
            for j in range(i + 1, NBLK):
                rj = slice(P * j, P * (j + 1))
                G[ri, rj] = U[ri, rj]
                G[rj, ri] = U[ri, rj].T
        xbar = S / N
        M = xbar.T @ xbar
        mean = xbar.mean(axis=0)
        within = (G - N * M) / tol
        between = N * (M - B * np.outer(mean, mean)) / tol
        w_h = within / np.sqrt(np.sum(np.diagonal(within) ** 2))
        b_h = between / np.sqrt(np.sum(np.diagonal(between) ** 2))
        if h == 0:
            w0, b0 = w_h, b_h
        else:
            loss = np.sum((w0 - w_h) ** 2) + np.sum((b0 - b_h) ** 2)
    return np.asarray(loss, dtype=np.float32)
